# revision 32
# baseline (speedup 1.0000x reference)
"""Trainium2 Bass kernel for nn_Decoder_34119220199683.

LAS-style attention LSTM decoder, teacher-forced, 300 sequential steps.
Sharding: data-parallel over batch N=64 -> 8 items per core on 8 cores.

Key design points:
- All big matmuls in bf16 (fp32 PSUM accumulate). Batch (8) is the
  stationary/M side, weights stream through the PE.
- LSTM activations use tanh only (sigmoid(x) = 0.5*tanh(x/2)+0.5), so the
  single ACT table set `exp_and_others` (exp+tanh) is loaded once, never
  switched. Scale factors are folded into host-side weights:
    * states stored doubled: C=2c, H=2h  -> cell update is
      C' = 0.5*(tanh_f+1)*C + (tanh_i+1)*tanh_g   (3 DVE ops)
      H' = (tanh_o+1)*tanh(0.5*C')                (1 DVE op)
    * W_hh/W_ih columns that consume h are pre-scaled by 0.5, key is
      pre-scaled by 0.5, output-embedding h2-rows pre-scaled by 0.5.
    * g-gate weight rows pre-scaled by 2 so one tanh(0.5*x) op covers
      all four gates.
- Softmax without max-subtraction (energies are small); masking is done by
  zeroing key/values beyond each item's len on the host and appending a
  0/1 "ones" column to values: denominator comes out of the ctx matmul.
- Items are sorted by len desc, rank r -> core r%8 slot r//8, so per-slot
  encoder-length budgets (compile-time, multiple of 512) clip attention
  work identically on all cores (SPMD-safe).
- All transposes (h1,h2,ctx,attn) go through DMA xbar transpose, keeping
  the PE free of transpose work.
"""

import sys

sys.path.insert(0, "/opt/trn_rl_repo")

import numpy as np
import ml_dtypes

import concourse.bacc as bacc
import concourse.bass as bass
import concourse.tile as tile
import concourse.mybir as mybir
from concourse.bass_utils import run_bass_kernel_spmd

F32 = mybir.dt.float32
BF16 = mybir.dt.bfloat16
AF = mybir.ActivationFunctionType
ALU = mybir.AluOpType
BF = ml_dtypes.bfloat16

# Problem constants
V, E, H, K, VS = 35, 256, 512, 128, 128
N, T, L = 64, 2000, 300
START = 33
NCORES = 8
NS = 8  # items (slots) per core
TCAP = 2048  # per-slot padded encoder length cap (multiple of 512)
UN = 15  # loop unroll factor (n_steps % UN == 0)


def build_program(T_slots, n_steps=L):
    """Build the bass/tile program for given per-slot T budgets."""
    VC = [t // 128 for t in T_slots]  # 128-chunks per slot (ctx)
    Toff = np.cumsum([0] + T_slots).tolist()  # key col offsets
    Voff = np.cumsum([0] + VC).tolist()  # val chunk offsets
    TK = Toff[-1]
    VCK = Voff[-1]
    ECmax = max((t + 511) // 512 for t in T_slots)

    def cw(s_, c_):  # width of slot s_'s energy chunk c_
        return max(0, min(512, T_slots[s_] - 512 * c_))

    nc = bacc.Bacc("TRN2", target_bir_lowering=False, debug=False,
                   num_devices=NCORES)

    # DRAM I/O
    d_key = nc.declare_dram_parameter("keyT", [128, TK], BF16, isOutput=False)
    d_val = nc.declare_dram_parameter("val", [128, VCK, 129], BF16, isOutput=False)
    d_ce = nc.declare_dram_parameter("ce", [128, 2, n_steps // UN, UN, NS], BF16,
                                     isOutput=False)
    d_w1 = nc.declare_dram_parameter("w1", [128, 7, 2048], BF16, isOutput=False)
    d_w2 = nc.declare_dram_parameter("w2", [128, 5, 512], BF16, isOutput=False)
    d_w3 = nc.declare_dram_parameter("w3", [128, 2, V], BF16, isOutput=False)
    d_out = nc.declare_dram_parameter("preds", [NS, n_steps // UN, UN, V], F32,
                                      isOutput=True)

    with tile.TileContext(nc) as tc:
        with (
            tc.tile_pool(name="const", bufs=1) as constp,
            tc.tile_pool(name="state", bufs=1) as statep,
            tc.tile_pool(name="work", bufs=2) as workp,
            tc.tile_pool(name="pg1", bufs=2, space="PSUM") as pg1,      # 2 banks
            tc.tile_pool(name="psmall", bufs=2, space="PSUM") as psmall,  # 2 banks
            tc.tile_pool(name="pe", bufs=2, space="PSUM") as pe,        # 4 banks
        ):
            # ---- constants in SBUF ----
            keyT = constp.tile([128, TK], BF16)
            val = constp.tile([128, VCK, 129], BF16)
            ce = constp.tile([128, 2, n_steps // UN, UN, NS], BF16)
            w1 = constp.tile([128, 7, 2048], BF16)
            w2 = constp.tile([128, 5, 512], BF16)
            w3 = constp.tile([128, 2, V], BF16)
            nc.sync.dma_start(out=keyT[:], in_=d_key[:])
            nc.sync.dma_start(out=val[:], in_=d_val[:])
            nc.sync.dma_start(out=ce[:], in_=d_ce[:])
            nc.sync.dma_start(out=w1[:], in_=d_w1[:])
            nc.sync.dma_start(out=w2[:], in_=d_w2[:])
            nc.sync.dma_start(out=w3[:], in_=d_w3[:])

            # ---- persistent state ----
            h1T = statep.tile([128, 4, 16], BF16)   # 2*h1 transposed (cols 0..7)
            h2T = statep.tile([128, 48], BF16)      # 2*h2 transposed (cols 0..15; rest 0)
            ctxTc = statep.tile([128, 16], BF16)    # ctx transposed; col s = slot s
            C1 = statep.tile([8, 512], F32)         # 2*c1
            C2 = statep.tile([8, 128], F32)         # 2*c2
            H1 = statep.tile([16, 512], BF16)       # 2*h1 (rows 8..15 zero)
            H2 = statep.tile([16, 128], BF16)
            ctx_nF = statep.tile([128, 2, 128], BF16)  # rows {0,32,64,96} valid
            attnF = statep.tile([128, 2, 2048], BF16)  # exp(energy), sparse rows
            attnC = statep.tile([16, 2048], BF16)   # compact: row s = slot s
            ctxC = statep.tile([16, 128], BF16)     # compact ctx: row s = slot s
            # [t, vc, q]: col q=s = attn of slot s, T-pos 128*vc+t
            attnT5 = statep.tile([128, 16, 16], BF16)
            tg1 = statep.tile([8, 2048], F32)       # tanh of gates1
            tc1 = statep.tile([8, 512], F32)        # tanh(c1)
            tg2 = statep.tile([8, 512], F32)
            tc2 = statep.tile([8, 128], F32)
            Pt = statep.tile([8, 512], F32)
            Qt = statep.tile([8, 512], F32)
            P2t = statep.tile([8, 128], F32)
            Q2t = statep.tile([8, 128], F32)
            rdenF = statep.tile([128, 2], F32)
            preds = statep.tile([8, n_steps // UN, UN, V], F32)

            for t_ in (h1T, h2T, ctxTc, C1, C2, H1, H2, attnF, attnT5, ctx_nF,
                       attnC, ctxC):
                nc.vector.memset(t_[:], 0.0)

            def step(i, u, dyn):
                # stage char embedding chunks (lhsT must have static offset)
                cest = workp.tile([128, 2, 1, NS], BF16, tag="cest")
                if dyn:
                    nc.vector.tensor_copy(out=cest,
                                          in_=ce[:, :, bass.ts(i, 1), u, :])
                else:
                    nc.vector.tensor_copy(out=cest, in_=ce[:, :, i : i + 1, u, :])

                # ---- LSTM1 gates, one PSUM bank (8,512) at a time ----
                # gate bank order [f, g, i, o] so the cell-update DVE chain
                # overlaps the remaining gate matmuls; ctx chunk LAST so
                # next-step gates can start before ctx transpose lands
                xc1 = [
                    cest[:, 0, 0, :], cest[:, 1, 0, :],
                    h1T[:, 0, 0:8], h1T[:, 1, 0:8], h1T[:, 2, 0:8], h1T[:, 3, 0:8],
                    ctxTc[:, 0:8],
                ]
                for b in range(4):
                    g1b = pg1.tile([8, 512], F32, tag="g1")
                    for j in range(7):
                        nc.tensor.matmul(
                            g1b, lhsT=xc1[j],
                            rhs=w1[:, j, b * 512 : (b + 1) * 512],
                            start=(j == 0), stop=(j == 6),
                        )
                    # tanh over this gate bank (g rows pre-doubled on host)
                    nc.scalar.activation(out=tg1[:, b * 512 : (b + 1) * 512],
                                         in_=g1b, func=AF.Tanh, scale=0.5)
                    if b == 0:
                        # Pt = (tanh_f + 1) * C1
                        nc.vector.scalar_tensor_tensor(
                            out=Pt, in0=tg1[:, 0:512], scalar=1.0, in1=C1,
                            op0=ALU.add, op1=ALU.mult)
                    elif b == 2:
                        # Qt = (tanh_i + 1) * tanh_g
                        nc.vector.scalar_tensor_tensor(
                            out=Qt, in0=tg1[:, 1024:1536], scalar=1.0,
                            in1=tg1[:, 512:1024],
                            op0=ALU.add, op1=ALU.mult)
                        # C1' = 0.5*Pt + Qt
                        nc.vector.scalar_tensor_tensor(
                            out=C1, in0=Pt, scalar=0.5, in1=Qt,
                            op0=ALU.mult, op1=ALU.add)
                        nc.scalar.activation(out=tc1, in_=C1, func=AF.Tanh,
                                             scale=0.5)
                # H1 = (to+1)*tanh(c1) = 2*h1
                nc.vector.scalar_tensor_tensor(
                    out=H1[0:8, :], in0=tg1[:, 1536:2048], scalar=1.0, in1=tc1,
                    op0=ALU.add, op1=ALU.mult)
                nc.sync.dma_start_transpose(out=h1T[:], in_=H1[:])

                # ---- LSTM2 gates: (8, 512) ----
                xc2 = [h1T[:, 0, 0:8], h1T[:, 1, 0:8], h1T[:, 2, 0:8],
                       h1T[:, 3, 0:8], h2T[:, 0:8]]
                g2b = psmall.tile([8, 512], F32, tag="small")
                for j in range(5):
                    nc.tensor.matmul(
                        g2b, lhsT=xc2[j], rhs=w2[:, j, :],
                        start=(j == 0), stop=(j == 4))
                nc.scalar.activation(out=tg2, in_=g2b, func=AF.Tanh, scale=0.5)
                nc.vector.scalar_tensor_tensor(
                    out=P2t, in0=tg2[:, 0:128], scalar=1.0, in1=C2,
                    op0=ALU.add, op1=ALU.mult)
                nc.vector.scalar_tensor_tensor(
                    out=Q2t, in0=tg2[:, 256:384], scalar=1.0, in1=tg2[:, 128:256],
                    op0=ALU.add, op1=ALU.mult)
                nc.vector.scalar_tensor_tensor(
                    out=C2, in0=P2t, scalar=0.5, in1=Q2t,
                    op0=ALU.mult, op1=ALU.add)
                nc.scalar.activation(out=tc2, in_=C2, func=AF.Tanh, scale=0.5)
                nc.vector.scalar_tensor_tensor(
                    out=H2[0:8, :], in0=tg2[:, 384:512], scalar=1.0, in1=tc2,
                    op0=ALU.add, op1=ALU.mult)
                nc.sync.dma_start_transpose(out=h2T[:, 0:16], in_=H2[:])

                # ---- attention: energy (PE) + exp (ACT) per 512-chunk ----
                for c in range(ECmax):
                    EAB = pe.tile([128, 2, 512], F32, tag="e")
                    for s in range(8):
                        w = cw(s, c)
                        if not w:
                            continue
                        g, jj = s // 4, s % 4
                        nc.tensor.matmul(
                            EAB[32 * jj : 32 * jj + 32, g, 0:w],
                            lhsT=h2T[:, s : s + 32],
                            rhs=keyT[:, Toff[s] + 512 * c : Toff[s] + 512 * c + w],
                            start=True, stop=True,
                            tile_position=(0, 32 * jj))
                    # exp (junk rows/cols harmless); compact rows
                    # {0,32,64,96}xg -> attnC row 4g+jj (gpsimd queue);
                    # transpose per chunk (sync queue)
                    wT = 0
                    for g in range(2):
                        wg = max([cw(s, c) for s in range(4 * g, 4 * g + 4)])
                        if not wg:
                            continue
                        wT = max(wT, wg)
                        nc.scalar.activation(
                            out=attnF[:, g, 512 * c : 512 * c + wg],
                            in_=EAB[:, g, 0:wg], func=AF.Exp)
                        nc.gpsimd.dma_start(
                            out=attnC[4 * g : 4 * g + 4, 512 * c : 512 * c + wg],
                            in_=attnF[0:97:32, g, 512 * c : 512 * c + wg])
                    nc.sync.dma_start_transpose(
                        out=attnT5[:, 4 * c : 4 * c + wT // 128, :],
                        in_=attnC[:, 512 * c : 512 * c + wT])

                # ---- ctx = attn @ [values | ones] per slot ----
                # group g1 (shortest slots, earliest-ready chunks) first;
                # its normalize/compact overlaps g0's matmuls
                cps = psmall.tile([128, 2, 129], F32, tag="small")
                for g in (1, 0):
                    for s in sorted(range(4 * g, 4 * g + 4),
                                    key=lambda s_: (VC[s_], s_)):
                        j = s % 4
                        for vc in range(VC[s]):
                            nc.tensor.matmul(
                                cps[32 * j : 32 * j + 1, g, :],
                                lhsT=attnT5[:, vc, s : s + 1],
                                rhs=val[:, Voff[s] + vc, :],
                                start=(vc == 0), stop=(vc == VC[s] - 1),
                                tile_position=(0, 32 * j))
                    # normalize this group (rows {0,32,64,96} valid)
                    nc.vector.reciprocal(out=rdenF[:, g : g + 1],
                                         in_=cps[:, g, 128:129])
                    nc.vector.tensor_scalar_mul(
                        out=ctx_nF[:, g, :], in0=cps[:, g, 0:128],
                        scalar1=rdenF[:, g : g + 1])
                    nc.gpsimd.dma_start(out=ctxC[4 * g : 4 * g + 4, :],
                                        in_=ctx_nF[0:97:32, g, :])
                nc.sync.dma_start_transpose(out=ctxTc[:], in_=ctxC[:])

                # ---- pred = [h2 | ctx] @ embT ----
                pp = pe.tile([8, 1, V], F32, tag="e")
                nc.tensor.matmul(pp[:, 0, :], lhsT=h2T[:, 0:8], rhs=w3[:, 0, :],
                                 start=True, stop=False)
                nc.tensor.matmul(pp[:, 0, :], lhsT=ctxTc[:, 0:8],
                                 rhs=w3[:, 1, :], start=False, stop=True)
                if dyn:
                    nc.vector.tensor_copy(out=preds[:, bass.ts(i, 1), u, :],
                                          in_=pp)
                else:
                    nc.vector.tensor_copy(out=preds[:, i : i + 1, u, :], in_=pp)

            with tc.For_i(0, n_steps // UN, 1) as i:
                for u in range(UN):
                    step(i, u, dyn=True)

            nc.sync.dma_start(out=d_out[:], in_=preds[:])

    nc.compile()
    return nc


def pack_inputs(key, values, lens, text, embedding,
                W_ih1, W_hh1, W_ih2, W_hh2, n_steps=L):
    """Host-side packing. Returns (in_maps, order, T_slots)."""
    key = np.asarray(key, np.float32)
    values = np.asarray(values, np.float32)
    lens = np.asarray(lens, np.int64)
    text = np.asarray(text, np.int64)
    emb = np.asarray(embedding, np.float32)

    order = np.argsort(-lens, kind="stable")
    # slot budgets from rank 8s (longest item in each slot), mult of 128
    T_slots = []
    for s in range(NS):
        lmax = int(lens[order[8 * s]])
        T_slots.append(min(TCAP, ((lmax + 127) // 128) * 128))
    VC = [t // 128 for t in T_slots]
    Toff = np.cumsum([0] + T_slots).tolist()
    Voff = np.cumsum([0] + VC).tolist()
    TK, VCK = Toff[-1], Voff[-1]

    # weights, gate order [f,g,i,o], g-rows doubled
    def reorder(W):
        i_, f_, g_, o_ = np.split(W, 4, axis=0)
        return np.concatenate([f_, 2.0 * g_, i_, o_], axis=0)

    W_ih1 = np.asarray(W_ih1, np.float32)
    # column order [ce | h1 | ctx] to match xc1 chunk order (ctx last)
    W1 = np.concatenate([W_ih1[:, :E], 0.5 * np.asarray(W_hh1, np.float32),
                         W_ih1[:, E:]], axis=1)  # (2048, 896)
    W1 = reorder(W1)
    w1 = W1.T.reshape(7, 128, 2048).transpose(1, 0, 2).astype(BF)  # (128,7,2048)

    W2 = np.concatenate([0.5 * np.asarray(W_ih2, np.float32),
                         0.5 * np.asarray(W_hh2, np.float32)], axis=1)  # (512, 640)
    W2 = reorder(W2)
    w2 = W2.T.reshape(5, 128, 512).transpose(1, 0, 2).astype(BF)  # (128,5,512)

    W3 = emb.T.copy()  # (256, 35)
    W3[:128] *= 0.5  # h2 rows (H2 = 2*h2)
    w3 = W3.reshape(2, 128, V).transpose(1, 0, 2).astype(BF)  # (128,2,V)

    emb_bf = emb.astype(BF).astype(np.float32)

    in_maps = []
    for c in range(NCORES):
        items = [int(order[8 * s + c]) for s in range(NS)]
        keyT = np.zeros((128, TK), np.float32)
        val = np.zeros((128, VCK, 129), np.float32)
        cearr = np.zeros((128, 2, n_steps, NS), np.float32)
        for s, it in enumerate(items):
            ln = int(lens[it])
            Ts = T_slots[s]
            lv = min(ln, Ts, T)
            keyT[:, Toff[s] : Toff[s] + lv] = 0.5 * key[it, :lv, :].T
            vv = values[it, : min(Ts, T), :]
            nch = VC[s]
            vbuf = np.zeros((nch * 128, 129), np.float32)
            vbuf[:lv, :128] = vv[:lv]
            vbuf[:lv, 128] = 1.0
            val[:, Voff[s] : Voff[s] + nch, :] = vbuf.reshape(nch, 128, 129).transpose(1, 0, 2)
            # char embeddings
            seq = np.empty((n_steps,), np.int64)
            seq[0] = START
            seq[1:] = text[it, : n_steps - 1]
            ce_i = emb_bf[seq]  # (n_steps, 256)
            cearr[:, 0, :, s] = ce_i[:, :128].T
            cearr[:, 1, :, s] = ce_i[:, 128:].T
        in_maps.append({
            "keyT": keyT.astype(BF),
            "val": val.astype(BF),
            "ce": cearr.astype(BF).reshape(128, 2, n_steps // UN, UN, NS),
            "w1": w1, "w2": w2, "w3": w3,
        })
    return in_maps, order, T_slots


_cache = {}


def kernel(key, values, lens, text, embedding, W_ih1, W_hh1, b_ih1, b_hh1,
           W_ih2, W_hh2, b_ih2, b_hh2, b_out, n_steps=L, trace=False):
    for b in (b_ih1, b_hh1, b_ih2, b_hh2, b_out):
        assert np.abs(np.asarray(b)).max() == 0.0, "nonzero biases unsupported"

    in_maps, order, T_slots = pack_inputs(
        key, values, lens, text, embedding, W_ih1, W_hh1, W_ih2, W_hh2,
        n_steps=n_steps)

    ck = (tuple(T_slots), n_steps)
    if ck not in _cache:
        _cache[ck] = build_program(T_slots, n_steps=n_steps)
    nc = _cache[ck]

    res = run_bass_kernel_spmd(nc, in_maps, core_ids=list(range(NCORES)),
                               trace=trace)
    out = np.empty((N, n_steps, V), np.float32)
    for c in range(NCORES):
        pr = res.results[c]["preds"].reshape(NS, n_steps, V)
        for s in range(NS):
            out[int(order[8 * s + c])] = pr[s]
    if trace:
        kernel._last_result = res
    return out



# revision 38
# speedup vs baseline: 1.2833x; 1.2833x over previous
"""Trainium2 Bass kernel for nn_Decoder_34119220199683.

LAS-style attention LSTM decoder, teacher-forced, 300 sequential steps.
Sharding: data-parallel over batch N=64 -> 8 items per core on 8 cores.

Key design points:
- All big matmuls in bf16 (fp32 PSUM accumulate). Batch (8) is the
  stationary/M side, weights stream through the PE.
- LSTM activations use tanh only (sigmoid(x) = 0.5*tanh(x/2)+0.5), so the
  single ACT table set `exp_and_others` (exp+tanh) is loaded once, never
  switched. Scale factors are folded into host-side weights:
    * states stored doubled: C=2c, H=2h  -> cell update is
      C' = 0.5*(tanh_f+1)*C + (tanh_i+1)*tanh_g   (3 DVE ops)
      H' = (tanh_o+1)*tanh(0.5*C')                (1 DVE op)
    * W_hh/W_ih columns that consume h are pre-scaled by 0.5, key is
      pre-scaled by 0.5, output-embedding h2-rows pre-scaled by 0.5.
    * g-gate weight rows pre-scaled by 2 so one tanh(0.5*x) op covers
      all four gates.
- Softmax without max-subtraction (energies are small); masking is done by
  zeroing key/values beyond each item's len on the host and appending a
  0/1 "ones" column to values: denominator comes out of the ctx matmul.
- Items are sorted by len desc, rank r -> core r%8 slot r//8, so per-slot
  encoder-length budgets (compile-time, multiple of 512) clip attention
  work identically on all cores (SPMD-safe).
- All transposes (h1,h2,ctx,attn) go through DMA xbar transpose, keeping
  the PE free of transpose work.
"""

import sys

sys.path.insert(0, "/opt/trn_rl_repo")

import numpy as np
import ml_dtypes

import concourse.bacc as bacc
import concourse.bass as bass
import concourse.tile as tile
import concourse.mybir as mybir
import concourse.masks as masks
from concourse.bass_utils import run_bass_kernel_spmd

F32 = mybir.dt.float32
BF16 = mybir.dt.bfloat16
AF = mybir.ActivationFunctionType
ALU = mybir.AluOpType
BF = ml_dtypes.bfloat16

# Problem constants
V, E, H, K, VS = 35, 256, 512, 128, 128
N, T, L = 64, 2000, 300
START = 33
NCORES = 8
NS = 8  # items (slots) per core
TCAP = 2048  # per-slot padded encoder length cap (multiple of 512)
UN = 15  # loop unroll factor (n_steps % UN == 0)


def build_program(T_slots, n_steps=L):
    """Build the bass/tile program for given per-slot T budgets."""
    VC = [t // 128 for t in T_slots]  # 128-chunks per slot (ctx)
    Toff = np.cumsum([0] + T_slots).tolist()  # key col offsets
    Voff = np.cumsum([0] + VC).tolist()  # val chunk offsets
    TK = Toff[-1]
    VCK = Voff[-1]
    ECmax = max((t + 511) // 512 for t in T_slots)

    def cw(s_, c_):  # width of slot s_'s energy chunk c_
        return max(0, min(512, T_slots[s_] - 512 * c_))

    nc = bacc.Bacc("TRN2", target_bir_lowering=False, debug=False,
                   num_devices=NCORES)

    # DRAM I/O
    d_key = nc.declare_dram_parameter("keyT", [128, TK], BF16, isOutput=False)
    d_val = nc.declare_dram_parameter("val", [128, VCK, 129], BF16, isOutput=False)
    d_ce = nc.declare_dram_parameter("ce", [128, 2, n_steps // UN, UN, NS], BF16,
                                     isOutput=False)
    d_w1 = nc.declare_dram_parameter("w1", [128, 7, 2048], BF16, isOutput=False)
    d_w2 = nc.declare_dram_parameter("w2", [128, 5, 512], BF16, isOutput=False)
    d_w3 = nc.declare_dram_parameter("w3", [128, 2, V], BF16, isOutput=False)
    d_out = nc.declare_dram_parameter("preds", [NS, n_steps // UN, UN, V], F32,
                                      isOutput=True)

    with tile.TileContext(nc) as tc:
        with (
            tc.tile_pool(name="const", bufs=1) as constp,
            tc.tile_pool(name="state", bufs=1) as statep,
            tc.tile_pool(name="work", bufs=2) as workp,
            tc.tile_pool(name="pg1", bufs=2, space="PSUM") as pg1,      # 2 banks
            tc.tile_pool(name="psmall", bufs=2, space="PSUM") as psmall,  # 2 banks
            tc.tile_pool(name="pe", bufs=2, space="PSUM") as pe,        # 4 banks
        ):
            # ---- constants in SBUF ----
            keyT = constp.tile([128, TK], BF16)
            val = constp.tile([128, VCK, 129], BF16)
            ce = constp.tile([128, 2, n_steps // UN, UN, NS], BF16)
            w1 = constp.tile([128, 7, 2048], BF16)
            w2 = constp.tile([128, 5, 512], BF16)
            w3 = constp.tile([128, 2, V], BF16)
            ident = constp.tile([128, 128], BF16)
            masks.make_identity(nc, ident[:])
            nc.sync.dma_start(out=keyT[:], in_=d_key[:])
            nc.sync.dma_start(out=val[:], in_=d_val[:])
            nc.sync.dma_start(out=ce[:], in_=d_ce[:])
            nc.sync.dma_start(out=w1[:], in_=d_w1[:])
            nc.sync.dma_start(out=w2[:], in_=d_w2[:])
            nc.sync.dma_start(out=w3[:], in_=d_w3[:])

            # ---- persistent state ----
            h1T = statep.tile([128, 4, 16], BF16)   # 2*h1 transposed (cols 0..7)
            h2T = statep.tile([128, 48], BF16)      # 2*h2 transposed (cols 0..15; rest 0)
            ctxTc = statep.tile([128, 16], BF16)    # ctx transposed; col s = slot s
            C1 = statep.tile([8, 512], F32)         # 2*c1
            C2 = statep.tile([8, 128], F32)         # 2*c2
            H1 = statep.tile([16, 512], BF16)       # 2*h1 (rows 8..15 zero)
            H2 = statep.tile([16, 128], BF16)
            ctx_nF = statep.tile([128, 2, 128], BF16)  # rows {0,32,64,96} valid
            attnF = statep.tile([128, 2, 2048], BF16)  # exp(energy), sparse rows
            attnC = statep.tile([16, 2048], BF16)   # compact: row s = slot s
            # [t, vc, q]: col q=s = attn of slot s, T-pos 128*vc+t
            attnT5 = statep.tile([128, 16, 16], BF16)
            tg1 = statep.tile([8, 2048], F32)       # tanh of gates1
            tc1 = statep.tile([8, 512], F32)        # tanh(c1)
            tg2 = statep.tile([8, 512], F32)
            tc2 = statep.tile([8, 128], F32)
            Pt = statep.tile([8, 512], F32)
            Qt = statep.tile([8, 512], F32)
            P2t = statep.tile([8, 128], F32)
            Q2t = statep.tile([8, 128], F32)
            rdenF = statep.tile([128, 2], F32)
            preds = statep.tile([8, n_steps // UN, UN, V], F32)

            for t_ in (h1T, h2T, ctxTc, C1, C2, H1, H2, attnF, attnT5, ctx_nF,
                       attnC):
                nc.vector.memset(t_[:], 0.0)

            def step(i, u, dyn):
                # stage char embedding chunks (lhsT must have static offset)
                cest = workp.tile([128, 2, 1, NS], BF16, tag="cest")
                if dyn:
                    nc.vector.tensor_copy(out=cest,
                                          in_=ce[:, :, bass.ts(i, 1), u, :])
                else:
                    nc.vector.tensor_copy(out=cest, in_=ce[:, :, i : i + 1, u, :])

                # ---- LSTM1 gates, one PSUM bank (8,512) at a time ----
                # gate bank order [f, g, i, o] so the cell-update DVE chain
                # overlaps the remaining gate matmuls; ctx chunk LAST so
                # next-step gates can start before ctx transpose lands
                xc1 = [
                    cest[:, 0, 0, :], cest[:, 1, 0, :],
                    h1T[:, 0, 0:8], h1T[:, 1, 0:8], h1T[:, 2, 0:8], h1T[:, 3, 0:8],
                    ctxTc[:, 0:8],
                ]
                for b in range(4):
                    g1b = pg1.tile([8, 512], F32, tag="g1")
                    for j in range(7):
                        nc.tensor.matmul(
                            g1b, lhsT=xc1[j],
                            rhs=w1[:, j, b * 512 : (b + 1) * 512],
                            start=(j == 0), stop=(j == 6),
                        )
                    # tanh over this gate bank (g rows pre-doubled on host)
                    nc.scalar.activation(out=tg1[:, b * 512 : (b + 1) * 512],
                                         in_=g1b, func=AF.Tanh, scale=0.5)
                    if b == 0:
                        # Pt = (tanh_f + 1) * C1
                        nc.vector.scalar_tensor_tensor(
                            out=Pt, in0=tg1[:, 0:512], scalar=1.0, in1=C1,
                            op0=ALU.add, op1=ALU.mult)
                    elif b == 2:
                        # Qt = (tanh_i + 1) * tanh_g
                        nc.vector.scalar_tensor_tensor(
                            out=Qt, in0=tg1[:, 1024:1536], scalar=1.0,
                            in1=tg1[:, 512:1024],
                            op0=ALU.add, op1=ALU.mult)
                        # C1' = 0.5*Pt + Qt
                        nc.vector.scalar_tensor_tensor(
                            out=C1, in0=Pt, scalar=0.5, in1=Qt,
                            op0=ALU.mult, op1=ALU.add)
                        nc.scalar.activation(out=tc1, in_=C1, func=AF.Tanh,
                                             scale=0.5)
                # H1 = (to+1)*tanh(c1) = 2*h1
                nc.vector.scalar_tensor_tensor(
                    out=H1[0:8, :], in0=tg1[:, 1536:2048], scalar=1.0, in1=tc1,
                    op0=ALU.add, op1=ALU.mult)
                # transpose H1 on the PE (idle here anyway); no DMA latency
                pt1 = pe.tile([128, 4, 16], BF16, tag="e")
                for cc in range(4):
                    nc.tensor.matmul(pt1[:, cc, :],
                                     lhsT=H1[:, 128 * cc : 128 * (cc + 1)],
                                     rhs=ident[0:16, 0:16], is_transpose=True)
                nc.vector.tensor_copy(out=h1T[:], in_=pt1)

                # ---- LSTM2 gates: (8, 512) ----
                xc2 = [h1T[:, 0, 0:8], h1T[:, 1, 0:8], h1T[:, 2, 0:8],
                       h1T[:, 3, 0:8], h2T[:, 0:8]]
                g2b = psmall.tile([8, 512], F32, tag="small")
                for j in range(5):
                    nc.tensor.matmul(
                        g2b, lhsT=xc2[j], rhs=w2[:, j, :],
                        start=(j == 0), stop=(j == 4))
                nc.scalar.activation(out=tg2, in_=g2b, func=AF.Tanh, scale=0.5)
                nc.vector.scalar_tensor_tensor(
                    out=P2t, in0=tg2[:, 0:128], scalar=1.0, in1=C2,
                    op0=ALU.add, op1=ALU.mult)
                nc.vector.scalar_tensor_tensor(
                    out=Q2t, in0=tg2[:, 256:384], scalar=1.0, in1=tg2[:, 128:256],
                    op0=ALU.add, op1=ALU.mult)
                nc.vector.scalar_tensor_tensor(
                    out=C2, in0=P2t, scalar=0.5, in1=Q2t,
                    op0=ALU.mult, op1=ALU.add)
                nc.scalar.activation(out=tc2, in_=C2, func=AF.Tanh, scale=0.5)
                nc.vector.scalar_tensor_tensor(
                    out=H2[0:8, :], in0=tg2[:, 384:512], scalar=1.0, in1=tc2,
                    op0=ALU.add, op1=ALU.mult)
                pt2 = pe.tile([128, 16], BF16, tag="e")
                nc.tensor.matmul(pt2[:], lhsT=H2[:], rhs=ident[0:16, 0:16],
                                 is_transpose=True)
                nc.vector.tensor_copy(out=h2T[:, 0:16], in_=pt2)

                # ---- attention: energy (PE) + exp (ACT) per 512-chunk ----
                for c in range(ECmax):
                    EAB = pe.tile([128, 2, 512], F32, tag="e")
                    for s in range(8):
                        w = cw(s, c)
                        if not w:
                            continue
                        g, jj = s // 4, s % 4
                        nc.tensor.matmul(
                            EAB[32 * jj : 32 * jj + 32, g, 0:w],
                            lhsT=h2T[:, s : s + 32],
                            rhs=keyT[:, Toff[s] + 512 * c : Toff[s] + 512 * c + w],
                            start=True, stop=True,
                            tile_position=(0, 32 * jj))
                    # exp (junk rows/cols harmless); compact rows
                    # {0,32,64,96}xg -> attnC row 4g+jj (gpsimd queue);
                    # transpose per chunk (sync queue)
                    wT = 0
                    for g in range(2):
                        wg = max([cw(s, c) for s in range(4 * g, 4 * g + 4)])
                        if not wg:
                            continue
                        wT = max(wT, wg)
                        nc.scalar.activation(
                            out=attnF[:, g, 512 * c : 512 * c + wg],
                            in_=EAB[:, g, 0:wg], func=AF.Exp)
                        nc.gpsimd.dma_start(
                            out=attnC[4 * g : 4 * g + 4, 512 * c : 512 * c + wg],
                            in_=attnF[0:97:32, g, 512 * c : 512 * c + wg])
                    nc.sync.dma_start_transpose(
                        out=attnT5[:, 4 * c : 4 * c + wT // 128, :],
                        in_=attnC[:, 512 * c : 512 * c + wT])

                # ---- ctx = attn @ [values | ones] per slot ----
                # group g1 (shortest slots, earliest-ready chunks) first;
                # its normalize/compact overlaps g0's matmuls
                cps = psmall.tile([128, 2, 129], F32, tag="small")
                for g in (1, 0):
                    for s in sorted(range(4 * g, 4 * g + 4),
                                    key=lambda s_: (VC[s_], s_)):
                        j = s % 4
                        for vc in range(VC[s]):
                            nc.tensor.matmul(
                                cps[32 * j : 32 * j + 1, g, :],
                                lhsT=attnT5[:, vc, s : s + 1],
                                rhs=val[:, Voff[s] + vc, :],
                                start=(vc == 0), stop=(vc == VC[s] - 1),
                                tile_position=(0, 32 * j))
                    # normalize this group (rows {0,32,64,96} valid)
                    nc.vector.reciprocal(out=rdenF[:, g : g + 1],
                                         in_=cps[:, g, 128:129])
                    nc.vector.tensor_scalar_mul(
                        out=ctx_nF[:, g, :], in0=cps[:, g, 0:128],
                        scalar1=rdenF[:, g : g + 1])
                # PE-transpose ctx (col 32j = slot 4g+j), gather to ctxTc
                ptc = pe.tile([128, 2, 128], BF16, tag="e")
                for g in range(2):
                    nc.tensor.matmul(ptc[:, g, :], lhsT=ctx_nF[:, g, :],
                                     rhs=ident[:], is_transpose=True)
                nc.vector.tensor_copy(out=ctxTc[:, 0:8],
                                      in_=ptc[:, :, 0:128:32])

                # ---- pred = [h2 | ctx] @ embT ----
                pp = pe.tile([8, 1, V], F32, tag="e")
                nc.tensor.matmul(pp[:, 0, :], lhsT=h2T[:, 0:8], rhs=w3[:, 0, :],
                                 start=True, stop=False)
                nc.tensor.matmul(pp[:, 0, :], lhsT=ctxTc[:, 0:8],
                                 rhs=w3[:, 1, :], start=False, stop=True)
                if dyn:
                    nc.vector.tensor_copy(out=preds[:, bass.ts(i, 1), u, :],
                                          in_=pp)
                else:
                    nc.vector.tensor_copy(out=preds[:, i : i + 1, u, :], in_=pp)

            with tc.For_i(0, n_steps // UN, 1) as i:
                for u in range(UN):
                    step(i, u, dyn=True)

            nc.sync.dma_start(out=d_out[:], in_=preds[:])

    nc.compile()
    return nc


def pack_inputs(key, values, lens, text, embedding,
                W_ih1, W_hh1, W_ih2, W_hh2, n_steps=L):
    """Host-side packing. Returns (in_maps, order, T_slots)."""
    key = np.asarray(key, np.float32)
    values = np.asarray(values, np.float32)
    lens = np.asarray(lens, np.int64)
    text = np.asarray(text, np.int64)
    emb = np.asarray(embedding, np.float32)

    order = np.argsort(-lens, kind="stable")
    # slot budgets from rank 8s (longest item in each slot), mult of 128
    T_slots = []
    for s in range(NS):
        lmax = int(lens[order[8 * s]])
        T_slots.append(min(TCAP, ((lmax + 127) // 128) * 128))
    VC = [t // 128 for t in T_slots]
    Toff = np.cumsum([0] + T_slots).tolist()
    Voff = np.cumsum([0] + VC).tolist()
    TK, VCK = Toff[-1], Voff[-1]

    # weights, gate order [f,g,i,o], g-rows doubled
    def reorder(W):
        i_, f_, g_, o_ = np.split(W, 4, axis=0)
        return np.concatenate([f_, 2.0 * g_, i_, o_], axis=0)

    W_ih1 = np.asarray(W_ih1, np.float32)
    # column order [ce | h1 | ctx] to match xc1 chunk order (ctx last)
    W1 = np.concatenate([W_ih1[:, :E], 0.5 * np.asarray(W_hh1, np.float32),
                         W_ih1[:, E:]], axis=1)  # (2048, 896)
    W1 = reorder(W1)
    w1 = W1.T.reshape(7, 128, 2048).transpose(1, 0, 2).astype(BF)  # (128,7,2048)

    W2 = np.concatenate([0.5 * np.asarray(W_ih2, np.float32),
                         0.5 * np.asarray(W_hh2, np.float32)], axis=1)  # (512, 640)
    W2 = reorder(W2)
    w2 = W2.T.reshape(5, 128, 512).transpose(1, 0, 2).astype(BF)  # (128,5,512)

    W3 = emb.T.copy()  # (256, 35)
    W3[:128] *= 0.5  # h2 rows (H2 = 2*h2)
    w3 = W3.reshape(2, 128, V).transpose(1, 0, 2).astype(BF)  # (128,2,V)

    emb_bf = emb.astype(BF).astype(np.float32)

    in_maps = []
    for c in range(NCORES):
        items = [int(order[8 * s + c]) for s in range(NS)]
        keyT = np.zeros((128, TK), np.float32)
        val = np.zeros((128, VCK, 129), np.float32)
        cearr = np.zeros((128, 2, n_steps, NS), np.float32)
        for s, it in enumerate(items):
            ln = int(lens[it])
            Ts = T_slots[s]
            lv = min(ln, Ts, T)
            keyT[:, Toff[s] : Toff[s] + lv] = 0.5 * key[it, :lv, :].T
            vv = values[it, : min(Ts, T), :]
            nch = VC[s]
            vbuf = np.zeros((nch * 128, 129), np.float32)
            vbuf[:lv, :128] = vv[:lv]
            vbuf[:lv, 128] = 1.0
            val[:, Voff[s] : Voff[s] + nch, :] = vbuf.reshape(nch, 128, 129).transpose(1, 0, 2)
            # char embeddings
            seq = np.empty((n_steps,), np.int64)
            seq[0] = START
            seq[1:] = text[it, : n_steps - 1]
            ce_i = emb_bf[seq]  # (n_steps, 256)
            cearr[:, 0, :, s] = ce_i[:, :128].T
            cearr[:, 1, :, s] = ce_i[:, 128:].T
        in_maps.append({
            "keyT": keyT.astype(BF),
            "val": val.astype(BF),
            "ce": cearr.astype(BF).reshape(128, 2, n_steps // UN, UN, NS),
            "w1": w1, "w2": w2, "w3": w3,
        })
    return in_maps, order, T_slots


_cache = {}


def kernel(key, values, lens, text, embedding, W_ih1, W_hh1, b_ih1, b_hh1,
           W_ih2, W_hh2, b_ih2, b_hh2, b_out, n_steps=L, trace=False):
    for b in (b_ih1, b_hh1, b_ih2, b_hh2, b_out):
        assert np.abs(np.asarray(b)).max() == 0.0, "nonzero biases unsupported"

    in_maps, order, T_slots = pack_inputs(
        key, values, lens, text, embedding, W_ih1, W_hh1, W_ih2, W_hh2,
        n_steps=n_steps)

    ck = (tuple(T_slots), n_steps)
    if ck not in _cache:
        _cache[ck] = build_program(T_slots, n_steps=n_steps)
    nc = _cache[ck]

    res = run_bass_kernel_spmd(nc, in_maps, core_ids=list(range(NCORES)),
                               trace=trace)
    out = np.empty((N, n_steps, V), np.float32)
    for c in range(NCORES):
        pr = res.results[c]["preds"].reshape(NS, n_steps, V)
        for s in range(NS):
            out[int(order[8 * s + c])] = pr[s]
    if trace:
        kernel._last_result = res
    return out

